# revision 1
# baseline (speedup 1.0000x reference)
"""BFP-quantized linear kernel for Trainium2, 8-core SPMD.

out = bfp_quantize(input) @ bfp_quantize(weight).T + bias
  input  [8192, 4608] f32, weight [4608, 4608] f32, bias [4608] f32
  BFP: groups of 36 contiguous elements (along rows), shared exponent
  from the group absmax, mantissas truncated toward zero to 8 bits.

Key facts exploited:
  * Quantized values are integers i in [-255, 255] times a power-of-two
    step, so they are EXACTLY representable in bf16 -> the matmul runs
    at bf16 speed with no additional error vs the f32 reference.
  * Truncation is done with pure integer bit-math on the fp32 encoding
    (HW float->int conversion rounds, so it cannot be used):
        q = x & (0x80000000 | ((143 + e - ex) << 23))   [mantissa mask]
        zeroed where e - ex >= 8
  * Sharding: rows of input (1024/core) and rows of weight (576/core).
    Each core quantizes its weight shard, PE-transposes it, and the
    bf16 [4608, 576] shards are AllGathered so every core has the full
    quantized+transposed weight at 1/8 the quantization cost.
  * Matmul computes out^T tiles (psum [o-block 128, n 512]) so the bias
    add rides the PSUM->SBUF drain on the scalar engine (per-partition
    bias AP). Host transposes the per-core [4608, 1024] result back.
"""

import numpy as np

import concourse.bass as bass
import concourse.mybir as mybir
import concourse.tile as tile
from concourse import bacc
from concourse import bass_utils
from concourse.masks import make_identity

N_CORES = 8
N_ROWS, K_IN, O_OUT = 8192, 4608, 4608
NSH = N_ROWS // N_CORES   # 1024 input rows per core
OSH = O_OUT // N_CORES    # 576 weight rows per core
GS = 36                   # BFP group size
KT = K_IN // 128          # 36 k tiles
NB = NSH // 128           # 8 n blocks per core
OB_TOT = O_OUT // 128     # 36 o blocks
CHUNK = 1152              # quantization column chunk (32 groups)
NCH = K_IN // CHUNK       # 4 chunks per row tile

# mask = x_bits & (0x80000000 + (143 + d) << 23) clears mantissa bits below
# 2^(e-7); d = e - ex is carried as (e_bits - ex_bits), already << 23.
C_MASK = -(2**31) + (143 << 23)   # fits int32
DD_LT8 = 8 << 23

F32 = mybir.dt.float32
BF16 = mybir.dt.bfloat16
I32 = mybir.dt.int32


def _emit_quant_chunk(nc, pool, src, qdst, rows, c0, neg1):
    """Quantize src[:rows, c0:c0+CHUNK] (f32) into qdst[:rows, c0:c0+CHUNK] (bf16).

    Probe-validated recipe (exact on HW):
      absmax -> e_byte;  ex = (bits>>23)&0xFF;  d = e_byte - ex
      L = min(d+16, 31); mask = -1 << L; qm = bits & mask
      q = f32(qm) * (d < 8)
    """
    g = CHUNK // GS
    xs = src[:rows, c0 : c0 + CHUNK]
    xg = xs.rearrange("p (g e) -> p g e", e=GS)
    xb = xs.bitcast(I32)

    absmax = pool.tile([128, g], F32, tag="absmax", name="absmax")
    nc.vector.tensor_reduce(
        out=absmax[:rows], in_=xg, axis=mybir.AxisListType.X,
        op=mybir.AluOpType.max, apply_absolute_value=True,
    )
    e_byte = pool.tile([128, g], I32, tag="e_byte", name="e_byte")
    nc.vector.tensor_scalar(
        out=e_byte[:rows], in0=absmax[:rows].bitcast(I32),
        scalar1=0x7F800000, scalar2=23,
        op0=mybir.AluOpType.bitwise_and,
        op1=mybir.AluOpType.logical_shift_right,
    )

    # d = e_byte - ex  (exponent bytes)
    d = pool.tile([128, CHUNK], I32, tag="qtmp1", name="d")
    nc.vector.tensor_scalar(
        out=d[:rows], in0=xb, scalar1=0x7F800000, scalar2=23,
        op0=mybir.AluOpType.bitwise_and,
        op1=mybir.AluOpType.logical_shift_right,
    )
    nc.vector.tensor_tensor(
        out=d[:rows].rearrange("p (g e) -> p g e", e=GS),
        in0=e_byte[:rows].unsqueeze(-1).broadcast_to([rows, g, GS]),
        in1=d[:rows].rearrange("p (g e) -> p g e", e=GS),
        op=mybir.AluOpType.subtract,
    )
    # zm = (d < 8) as 1.0/0.0
    zm = pool.tile([128, CHUNK], F32, tag="qtmp2", name="zm")
    nc.vector.tensor_scalar(
        out=zm[:rows], in0=d[:rows], scalar1=8, scalar2=None,
        op0=mybir.AluOpType.is_lt,
    )
    # L = min(d + 16, 31) ; mask = -1 << L  (in place over d)
    nc.vector.tensor_scalar(
        out=d[:rows], in0=d[:rows], scalar1=16, scalar2=31,
        op0=mybir.AluOpType.add, op1=mybir.AluOpType.min,
    )
    nc.vector.tensor_tensor(
        out=d[:rows], in0=neg1[:rows].broadcast_to([rows, CHUNK]), in1=d[:rows],
        op=mybir.AluOpType.logical_shift_left,
    )
    # int32 bitwise ops are DVE-only; the final f32 multiply offloads to
    # GpSimd (otherwise idle) to shorten the DVE chain gating matmul start.
    qm = pool.tile([128, CHUNK], I32, tag="qtmp3", name="qm")
    nc.vector.tensor_tensor(
        out=qm[:rows], in0=xb, in1=d[:rows], op=mybir.AluOpType.bitwise_and,
    )
    nc.gpsimd.tensor_tensor(
        out=qdst[:rows, c0 : c0 + CHUNK], in0=qm[:rows].bitcast(F32),
        in1=zm[:rows], op=mybir.AluOpType.mult,
    )


def emit_kernel(tc, nc, x_d, w_d, b_d, o_d):
    with (
        tc.tile_pool(name="dram", bufs=1, space="DRAM") as dpool,
        tc.tile_pool(name="consts", bufs=1) as cpool,
        tc.tile_pool(name="stage", bufs=2) as spool,
        tc.tile_pool(name="qtmps", bufs=1) as tpool,
        tc.tile_pool(name="qnat", bufs=2) as qpool,
        tc.tile_pool(name="qxt", bufs=1) as xtpool,
        tc.tile_pool(name="wstream", bufs=38) as wpool,
        tc.tile_pool(name="tstage", bufs=4) as tspool,
        tc.tile_pool(name="outs", bufs=2) as opool,
        tc.tile_pool(name="pmm", bufs=5, space="PSUM") as pmm,
        tc.tile_pool(name="ptp", bufs=3, space="PSUM") as ptp,
    ):
        ident = cpool.tile([128, 128], BF16, name="ident")
        make_identity(nc, ident[:])
        neg1 = cpool.tile([128, 1], I32, name="neg1")
        nc.vector.memset(neg1[:], -1)
        # biasT[p, ob] = bias[ob*128 + p]
        biasT = cpool.tile([128, OB_TOT], F32, name="biasT")
        nc.sync.dma_start(
            out=biasT[:], in_=b_d.rearrange("(o p) -> p o", p=128)
        )

        # ---------- weight shard: quantize + transpose + bounce ----------
        # k-quarter-major so each quarter's AllGather launches early and
        # pipelines with the remaining quantization.
        HALF = K_IN // 2
        NQ = 4
        QW = K_IN // NQ  # 1152 k per quarter = 9 k-tiles
        w_tiles = [(i * 128, min(128, OSH - i * 128)) for i in range((OSH + 127) // 128)]
        qw_boun = [
            dpool.tile([QW, OSH], BF16, name=f"qw_boun{q}") for q in range(NQ)
        ]
        qwt_g = [
            dpool.tile(
                [N_CORES * QW, OSH], BF16, addr_space="Shared", name=f"qwt_g{q}"
            )
            for q in range(NQ)
        ]
        for q in range(NQ):
            for r0, rows in w_tiles:
                wtile = spool.tile([128, QW], F32, tag="stage", name="wtile")
                nc.sync.dma_start(
                    out=wtile[:rows], in_=w_d[r0 : r0 + rows, q * QW : (q + 1) * QW]
                )
                qw = qpool.tile([128, QW], BF16, tag="qnat", name="qw")
                _emit_quant_chunk(nc, tpool, wtile, qw, rows, 0, neg1)
                for ktl in range(QW // 128):
                    pt = ptp.tile([128, 128], BF16, tag="tp", name="pt")
                    nc.tensor.transpose(
                        pt[:, :rows], qw[:rows, ktl * 128 : (ktl + 1) * 128],
                        ident[:rows, :rows],
                    )
                    st = tspool.tile([128, 128], BF16, tag="ts", name="st")
                    nc.scalar.copy(st[:, :rows], pt[:, :rows])
                    nc.sync.dma_start(
                        out=qw_boun[q][ktl * 128 : (ktl + 1) * 128, r0 : r0 + rows],
                        in_=st[:, :rows],
                    )
            nc.gpsimd.collective_compute(
                "AllGather",
                mybir.AluOpType.bypass,
                replica_groups=[list(range(N_CORES))],
                ins=[qw_boun[q][:].opt()],
                outs=[qwt_g[q][:].opt()],
            )

        # ---------- input shard: quantize + PE transpose into resident qxT ----------
        qxT = [
            xtpool.tile([128, NSH], BF16, name=f"qxT{kt}") for kt in range(KT)
        ]
        for nb in range(NB):
            for h in range(2):
                k0 = h * HALF
                xtile = spool.tile([128, HALF], F32, tag="stage", name="xtile")
                nc.sync.dma_start(
                    out=xtile[:], in_=x_d[nb * 128 : (nb + 1) * 128, k0 : k0 + HALF]
                )
                qx = qpool.tile([128, HALF], BF16, tag="qnat", name="qx")
                for ch in range(NCH // 2):
                    _emit_quant_chunk(nc, tpool, xtile, qx, 128, ch * CHUNK, neg1)
                for ktl in range(KT // 2):
                    kt = h * (KT // 2) + ktl
                    pt = ptp.tile([128, 128], BF16, tag="tp", name="pt")
                    nc.tensor.transpose(
                        pt[:], qx[:, ktl * 128 : (ktl + 1) * 128], ident[:]
                    )
                    nc.scalar.copy(qxT[kt][:, nb * 128 : (nb + 1) * 128], pt[:])

        # ---------- matmul: psum[o128, n512] over 36 k tiles ----------
        # gathered layout: shard c occupies rows [c*K_IN, (c+1)*K_IN) as [4608, 576]
        for og in range(4):  # o-group = 1152 cols = shards 2og, 2og+1
            wq = []
            for kt in range(KT):
                q, ktl = kt // (QW // 128), kt % (QW // 128)
                wqt = wpool.tile([128, 2 * OSH], BF16, tag="wq", name="wqt")
                for h in range(2):
                    c = 2 * og + h
                    nc.sync.dma_start(
                        out=wqt[:, h * OSH : (h + 1) * OSH],
                        in_=qwt_g[q][c * QW + ktl * 128 : c * QW + (ktl + 1) * 128, :],
                    )
                wq.append(wqt)
            # two n-half sweeps per og: sweep A (cols 0:512, input rows 0-511)
            # only needs the first half of the x shard quantized, so matmuls
            # start while the second half is still being quantized.
            for half in range(2):
                n0 = half * 512
                for obl in range(9):
                    ob = og * 9 + obl
                    ps = pmm.tile([128, 512], F32, tag="mm", name="ps")
                    for kt in range(KT):
                        lhsT = wq[kt][:, obl * 128 : (obl + 1) * 128]
                        nc.tensor.matmul(
                            ps[:], lhsT, qxT[kt][:, n0 : n0 + 512],
                            start=(kt == 0), stop=(kt == KT - 1),
                        )
                    ot = opool.tile([128, 512], F32, tag="ot", name="ot")
                    nc.scalar.activation(
                        ot[:], ps[:],
                        mybir.ActivationFunctionType.Identity,
                        bias=biasT[:, ob : ob + 1], scale=1.0,
                    )
                    nc.sync.dma_start(
                        out=o_d[ob * 128 : (ob + 1) * 128, n0 : n0 + 512], in_=ot[:]
                    )


_CACHED_NC = None


def _build():
    global _CACHED_NC
    if _CACHED_NC is not None:
        return _CACHED_NC
    nc = bacc.Bacc(
        "TRN2", target_bir_lowering=False, debug=False, num_devices=N_CORES
    )
    x_d = nc.dram_tensor("x", [NSH, K_IN], F32, kind="ExternalInput").ap()
    w_d = nc.dram_tensor("w", [OSH, K_IN], F32, kind="ExternalInput").ap()
    b_d = nc.dram_tensor("b", [O_OUT], F32, kind="ExternalInput").ap()
    o_d = nc.dram_tensor("o", [O_OUT, NSH], F32, kind="ExternalOutput").ap()
    with tile.TileContext(nc) as tc:
        emit_kernel(tc, nc, x_d, w_d, b_d, o_d)
    nc.compile()
    _CACHED_NC = nc
    return nc


def _ensure_axon_hooks_importable():
    # bass_utils imports antenv.axon_hooks when tracing is requested; the
    # slim agent image lacks it. Provide a no-op so a stray BASS_TRACE env
    # degrades to "no trace" instead of crashing.
    import sys
    import types

    if "antenv.axon_hooks" not in sys.modules:
        try:
            import antenv.axon_hooks  # noqa: F401
        except ImportError:
            mod = types.ModuleType("antenv.axon_hooks")
            mod.get_axon_ntff_profile_hook = lambda: None
            mod.set_axon_ntff_profile_hook = lambda h: None
            sys.modules["antenv.axon_hooks"] = mod


def run_on_hw(input, weight, bias, trace=False):
    _ensure_axon_hooks_importable()
    nc = _build()
    in_maps = []
    for c in range(N_CORES):
        in_maps.append(
            {
                "x": np.ascontiguousarray(input[c * NSH : (c + 1) * NSH]),
                "w": np.ascontiguousarray(weight[c * OSH : (c + 1) * OSH]),
                "b": np.ascontiguousarray(bias),
            }
        )
    res = bass_utils.run_bass_kernel_spmd(
        nc, in_maps, core_ids=list(range(N_CORES)), trace=trace
    )
    out = np.empty((N_ROWS, O_OUT), dtype=np.float32)
    for c in range(N_CORES):
        out[c * NSH : (c + 1) * NSH] = res.results[c]["o"].T
    return out, res


def kernel(input, weight, bias):
    out, _ = run_on_hw(
        np.asarray(input, dtype=np.float32),
        np.asarray(weight, dtype=np.float32),
        np.asarray(bias, dtype=np.float32),
    )
    return out



# revision 11
# speedup vs baseline: 1.1709x; 1.1709x over previous
"""BFP-quantized linear kernel for Trainium2, 8-core SPMD.

out = bfp_quantize(input) @ bfp_quantize(weight).T + bias
  input  [8192, 4608] f32, weight [4608, 4608] f32, bias [4608] f32
  BFP: groups of 36 contiguous elements (along rows), shared exponent
  from the group absmax, mantissas truncated toward zero to 8 bits.

Design (v2):
  * Quantization via two custom DVE ops (3 DVE passes total, bit-exact):
      op1 ANT_BFP_FLOORMAG: za = floor(|x|/step)*step using the
          1.5*2^23 magic-constant RNE trick + floor correction (7 ALU ops)
      op2 ANT_COPYSIGN:     q  = za | (x & -0.0), bf16 out (2 ALU ops)
    step = 2^(e-7) comes from absmax-reduce + 2 tiny bit ops per group.
    step=0 (all-zero group) degrades to identity, matching the reference.
  * Sharding: input rows (1024/core) + weight rows (576/core). Each core
    quantizes + PE-transposes its weight shard per K-quarter, AllGathers
    the bf16 [1152, 576] shards into a column-concatenated [1152, 4608]
    view so every o-column block is one contiguous DMA later.
  * Matmul orientation: psum[n=128, o=512]; lhsT = qxT k-tiles (resident
    SBUF), rhs = streamed weight "og-sets" (512 o-columns x 36 k-tiles).
    Output lands as [1024, 4608] per core - no host transpose.
  * Bias rides a K=2 leading matmul per chain: ones[2,128]^T @ [bh;bl]
    where bias = bh + bl splits into two bf16s (error ~4e-8).
  * Pipeline: W-quarters (quant+transpose+AG) -> per-nb: quant x(nb),
    transpose, chains(og0, nb) -> og 1..8 back-to-back chains with
    og-set prefetch via a 48-buf pool.
"""

import numpy as np

import concourse.bass as bass
import concourse.mybir as mybir
import concourse.tile as tile
from concourse import bacc
from concourse import bass_utils
from concourse.masks import make_identity

N_CORES = 8
N_ROWS, K_IN, O_OUT = 8192, 4608, 4608
NSH = N_ROWS // N_CORES   # 1024 input rows per core
OSH = O_OUT // N_CORES    # 576 weight rows per core
GS = 36                   # BFP group size
KT = K_IN // 128          # 36 k tiles
NB = NSH // 128           # 8 n blocks per core
QW = K_IN // 4            # 1152 k per AG quarter = 9 k-tiles
OGW = 512                 # o-columns per matmul chain
NOG = O_OUT // OGW        # 9 o groups
XHW = K_IN // 2           # 2304: x half-tile width for transposes
XQW = QW                  # 1152: x quant chunk width (32 groups)
BIG = 12582912.0          # 1.5 * 2**23

F32 = mybir.dt.float32
BF16 = mybir.dt.bfloat16
I32 = mybir.dt.int32


# --------------------------------------------------------------------------
# Custom DVE ops (registered once per process; additive, name-keyed)
# --------------------------------------------------------------------------

def _register_dve_op(name, spec):
    from concourse import dve_ops as _ops
    from concourse.dve_spec import lower
    from concourse.dve_uop import DveOpSpec

    for op in _ops.OPS:
        if op.name == name:
            return op
    row = 1 + len(_ops.OPS)
    uops = lower(spec, ver="v3")
    sha = DveOpSpec(name=name, opcode=row, uops=uops, rd1_en=True).sha("v3")
    op = _ops.DveOp(name, spec, subdim=False, uops_sha={"v3": sha})
    _ops.OPS.append(op)
    _ops._SUB_OPCODE_FOR_NAME[name] = row
    _ops.CUSTOM_DVE_SPECS[name] = spec
    return op


def register_quant_ops():
    from concourse.dve_spec import Spec, Src0, Src1, C0, C1, Bin
    from concourse.dve_uop import AluOp as A

    ax = Bin(A.ABSOLUTE_VALUE, Src0, Src0)
    b = Src1 * C0                      # C0 = 12582912.0 = 1.5 * 2^23
    y = (ax + b) - b                   # RNE of |x| to step grid (exact)
    c = y > ax
    za = y - c * Src1                  # floor correction

    def _ref1(in0, in1, s0, s1, imm2):
        axn = np.abs(in0).astype(np.float32)
        bn = (in1 * s0).astype(np.float32)
        yn = ((axn + bn) - bn).astype(np.float32)
        return (yn - (yn > axn) * in1).astype(np.float32)

    op1 = _register_dve_op("ANT_BFP_FLOORMAG", Spec(body=za, reference=_ref1))

    sb = Bin(A.BITWISE_AND, Src1, C1)  # C1 = -0.0 -> sign bit of x
    body2 = Bin(A.BITWISE_OR, Src0, sb)

    def _ref2(in0, in1, s0, s1, imm2):
        return np.copysign(in0, in1).astype(np.float32)

    op2 = _register_dve_op("ANT_COPYSIGN", Spec(body=body2, reference=_ref2))
    return op1, op2


# --------------------------------------------------------------------------
# Kernel emission
# --------------------------------------------------------------------------

def _emit_quant(nc, ops, gpool, zapool, src, qdst, rows, width):
    """src[:rows, :width] f32 -> qdst[:rows, :width] bf16 (36-elem groups)."""
    op1, op2 = ops
    g = width // GS
    xg = src[:rows, :width].rearrange("p (g e) -> p g e", e=GS)
    absmax = gpool.tile([128, g], F32, tag="absmax", name="absmax")
    nc.vector.tensor_reduce(
        out=absmax[:rows], in_=xg, axis=mybir.AxisListType.X,
        op=mybir.AluOpType.max, apply_absolute_value=True,
    )
    step = gpool.tile([128, g], F32, tag="step", name="step")
    nc.vector.tensor_scalar(
        out=step[:rows].bitcast(I32), in0=absmax[:rows].bitcast(I32),
        scalar1=0x7F800000, scalar2=None, op0=mybir.AluOpType.bitwise_and,
    )
    nc.vector.tensor_scalar(
        out=step[:rows], in0=step[:rows], scalar1=2.0 ** -7, scalar2=None,
        op0=mybir.AluOpType.mult,
    )
    za = zapool.tile([128, width], F32, tag=f"za{width}", name="za")
    nc.vector._custom_dve(
        op1,
        out=za[:rows].rearrange("p (g e) -> p g e", e=GS),
        in0=xg,
        in1=step[:rows].unsqueeze(-1).broadcast_to([rows, g, GS]),
        s0=BIG,
    )
    nc.vector._custom_dve(
        op2,
        out=qdst[:rows, :width],
        in0=za[:rows],
        in1=src[:rows, :width],
        s1=-0.0,
    )


def emit_kernel(tc, nc, ops, x_d, w_d, b_d, o_d):
    w_tiles = [(i * 128, min(128, OSH - i * 128)) for i in range((OSH + 127) // 128)]
    with (
        tc.tile_pool(name="dram", bufs=1, space="DRAM") as dpool,
        tc.tile_pool(name="consts", bufs=1) as cpool,
        tc.tile_pool(name="grp", bufs=3) as gpool,
        tc.tile_pool(name="za", bufs=2) as zapool,
        tc.tile_pool(name="wstage", bufs=2) as wspool,
        tc.tile_pool(name="qw", bufs=6) as qwpool,
        tc.tile_pool(name="stw", bufs=2) as stwpool,
        tc.tile_pool(name="xstage", bufs=2) as xspool,
        tc.tile_pool(name="qx", bufs=2) as qxpool,
        tc.tile_pool(name="qxT", bufs=1) as xtpool,
        tc.tile_pool(name="ogset", bufs=48) as ogpool,
        tc.tile_pool(name="outs", bufs=3) as opool,
        tc.tile_pool(name="pmm", bufs=4, space="PSUM") as pmm,
        tc.tile_pool(name="ptp", bufs=3, space="PSUM") as ptp,
    ):
        ident = cpool.tile([128, 128], BF16, name="ident")
        make_identity(nc, ident[:])
        ones2 = cpool.tile([2, 128], BF16, name="ones2")
        nc.vector.memset(ones2[:], 1.0)

        # ---------- bias: split into bh + bl (bf16 pair), row layout ----------
        bias_rs = cpool.tile([128, GS], F32, name="bias_rs")
        nc.sync.dma_start(out=bias_rs[:], in_=b_d.rearrange("(p o) -> p o", o=GS))
        bh_rs = cpool.tile([128, GS], BF16, name="bh_rs")
        nc.scalar.copy(bh_rs[:], bias_rs[:])
        bhf_rs = cpool.tile([128, GS], F32, name="bhf_rs")
        nc.scalar.copy(bhf_rs[:], bh_rs[:])
        bl_rs = cpool.tile([128, GS], BF16, name="bl_rs")
        nc.vector.tensor_tensor(
            out=bl_rs[:], in0=bias_rs[:], in1=bhf_rs[:],
            op=mybir.AluOpType.subtract,
        )
        # reshape [128, 36] -> [1, 4608] rows via DRAM bounce
        bh_dr = dpool.tile([O_OUT], BF16, name="bh_dr")
        bl_dr = dpool.tile([O_OUT], BF16, name="bl_dr")
        nc.sync.dma_start(out=bh_dr.rearrange("(p o) -> p o", o=GS), in_=bh_rs[:])
        nc.sync.dma_start(out=bl_dr.rearrange("(p o) -> p o", o=GS), in_=bl_rs[:])
        bias2 = cpool.tile([2, O_OUT], BF16, name="bias2")
        nc.sync.dma_start(out=bias2[0:1, :], in_=bh_dr.rearrange("(a o) -> a o", a=1))
        nc.sync.dma_start(out=bias2[1:2, :], in_=bl_dr.rearrange("(a o) -> a o", a=1))

        # ---------- weight shard: quantize + transpose + bounce + AG ----------
        # qwt_g[q] is [8*QW, OSH]: shard c occupies rows [c*QW, (c+1)*QW).
        qw_boun = [dpool.tile([QW, OSH], BF16, name=f"qw_boun{q}") for q in range(4)]
        qwt_g = [
            dpool.tile([N_CORES * QW, OSH], BF16, addr_space="Shared", name=f"qwt_g{q}")
            for q in range(4)
        ]
        for q in range(4):
            qws = []
            for r0, rows in w_tiles:
                wtile = wspool.tile([128, QW], F32, tag="wstage", name="wtile")
                nc.sync.dma_start(
                    out=wtile[:rows], in_=w_d[r0 : r0 + rows, q * QW : (q + 1) * QW]
                )
                qw = qwpool.tile([128, QW], BF16, tag="qw", name="qw")
                _emit_quant(nc, ops, gpool, zapool, wtile, qw, rows, QW)
                qws.append(qw)
            for ktl in range(9):
                stw = stwpool.tile([128, OSH], BF16, tag="stw", name="stw")
                for (r0, rows), qw in zip(w_tiles, qws):
                    pt = ptp.tile([128, 128], BF16, tag="tp", name="pt")
                    nc.tensor.transpose(
                        pt[:, :rows], qw[:rows, ktl * 128 : (ktl + 1) * 128],
                        ident[:rows, :rows],
                    )
                    nc.scalar.copy(stw[:, r0 : r0 + rows], pt[:, :rows])
                nc.sync.dma_start(
                    out=qw_boun[q][ktl * 128 : (ktl + 1) * 128, :], in_=stw[:]
                )
            nc.gpsimd.collective_compute(
                "AllGather",
                mybir.AluOpType.bypass,
                replica_groups=[list(range(N_CORES))],
                ins=[qw_boun[q][:].opt()],
                outs=[qwt_g[q][:].opt()],
            )

        # ---------- og-set loads (1-2 DMAs per k-tile: shard-span splits) ----
        og_tiles = {}

        def load_ogset(og):
            tl = []
            o0 = og * OGW
            spans = []
            o = o0
            while o < o0 + OGW:
                c = o // OSH
                hi = min((c + 1) * OSH, o0 + OGW)
                spans.append((c, o, hi))
                o = hi
            for kt in range(KT):
                q, ktl = divmod(kt, 9)
                t = ogpool.tile([128, OGW], BF16, tag="og", name=f"og{og}_{kt}")
                for c, lo, hi in spans:
                    nc.sync.dma_start(
                        out=t[:, lo - o0 : hi - o0],
                        in_=qwt_g[q][
                            c * QW + ktl * 128 : c * QW + (ktl + 1) * 128,
                            lo - c * OSH : hi - c * OSH,
                        ],
                    )
                tl.append(t)
            og_tiles[og] = tl

        # ---------- x quant (per nb row-tile, 2 half-K chunks) ---------------
        qxT = [xtpool.tile([128, NSH], BF16, name=f"qxT{kt}") for kt in range(KT)]

        def emit_xquant(nb):
            for h in range(2):
                qx = qxpool.tile([128, XHW], BF16, tag="qx", name="qx")
                for ch in range(XHW // XQW):
                    xtile = xspool.tile([128, XQW], F32, tag="xstage", name="xtile")
                    c0 = h * XHW + ch * XQW
                    nc.sync.dma_start(
                        out=xtile[:],
                        in_=x_d[nb * 128 : (nb + 1) * 128, c0 : c0 + XQW],
                    )
                    _emit_quant(
                        nc, ops, gpool, zapool, xtile,
                        qx[:, ch * XQW : (ch + 1) * XQW], 128, XQW,
                    )
                for ktl in range(KT // 2):
                    kt = h * (KT // 2) + ktl
                    pt = ptp.tile([128, 128], BF16, tag="tp", name="pt")
                    nc.tensor.transpose(
                        pt[:], qx[:, ktl * 128 : (ktl + 1) * 128], ident[:]
                    )
                    nc.scalar.copy(qxT[kt][:, nb * 128 : (nb + 1) * 128], pt[:])

        # ---------- matmul chain: psum[n=128, o=512] -------------------------
        def emit_chain(og, nb):
            ps = pmm.tile([128, OGW], F32, tag="mm", name="ps")
            nc.tensor.matmul(
                ps[:], ones2[:], bias2[:, og * OGW : (og + 1) * OGW],
                start=True, stop=False,
            )
            tl = og_tiles[og]
            for kt in range(KT):
                nc.tensor.matmul(
                    ps[:],
                    qxT[kt][:, nb * 128 : (nb + 1) * 128],
                    tl[kt][:],
                    start=False, stop=(kt == KT - 1),
                )
            ot = opool.tile([128, OGW], F32, tag="ot", name="ot")
            nc.scalar.copy(ot[:], ps[:])
            nc.scalar.dma_start(
                out=o_d[nb * 128 : (nb + 1) * 128, og * OGW : (og + 1) * OGW],
                in_=ot[:],
            )

        # ---------- schedule ------------------------------------------------
        load_ogset(0)
        for nb in range(NB):
            emit_xquant(nb)
            emit_chain(0, nb)
        for og in range(1, NOG):
            load_ogset(og)
            for nb in range(NB):
                emit_chain(og, nb)
            del og_tiles[og - 1]


_CACHED_NC = None


def _build():
    global _CACHED_NC
    if _CACHED_NC is not None:
        return _CACHED_NC
    ops = register_quant_ops()
    nc = bacc.Bacc(
        "TRN2", target_bir_lowering=False, debug=False, num_devices=N_CORES
    )
    x_d = nc.dram_tensor("x", [NSH, K_IN], F32, kind="ExternalInput").ap()
    w_d = nc.dram_tensor("w", [OSH, K_IN], F32, kind="ExternalInput").ap()
    b_d = nc.dram_tensor("b", [O_OUT], F32, kind="ExternalInput").ap()
    o_d = nc.dram_tensor("o", [NSH, O_OUT], F32, kind="ExternalOutput").ap()
    with tile.TileContext(nc) as tc:
        emit_kernel(tc, nc, ops, x_d, w_d, b_d, o_d)
    nc.compile()
    _CACHED_NC = nc
    return nc


def _ensure_axon_hooks_importable():
    # bass_utils imports antenv.axon_hooks when tracing is requested; the
    # slim agent image lacks it. Provide a no-op so a stray BASS_TRACE env
    # degrades to "no trace" instead of crashing.
    import sys
    import types

    if "antenv.axon_hooks" not in sys.modules:
        try:
            import antenv.axon_hooks  # noqa: F401
        except ImportError:
            mod = types.ModuleType("antenv.axon_hooks")
            mod.get_axon_ntff_profile_hook = lambda: None
            mod.set_axon_ntff_profile_hook = lambda h: None
            sys.modules["antenv.axon_hooks"] = mod


def run_on_hw(input, weight, bias, trace=False):
    _ensure_axon_hooks_importable()
    nc = _build()
    in_maps = []
    for c in range(N_CORES):
        in_maps.append(
            {
                "x": np.ascontiguousarray(input[c * NSH : (c + 1) * NSH]),
                "w": np.ascontiguousarray(weight[c * OSH : (c + 1) * OSH]),
                "b": np.ascontiguousarray(bias),
            }
        )
    res = bass_utils.run_bass_kernel_spmd(
        nc, in_maps, core_ids=list(range(N_CORES)), trace=trace
    )
    out = np.empty((N_ROWS, O_OUT), dtype=np.float32)
    for c in range(N_CORES):
        out[c * NSH : (c + 1) * NSH] = res.results[c]["o"]
    return out, res


def kernel(input, weight, bias):
    out, _ = run_on_hw(
        np.asarray(input, dtype=np.float32),
        np.asarray(weight, dtype=np.float32),
        np.asarray(bias, dtype=np.float32),
    )
    return out


# revision 14
# speedup vs baseline: 1.1910x; 1.0172x over previous
"""BFP-quantized linear kernel for Trainium2, 8-core SPMD.

out = bfp_quantize(input) @ bfp_quantize(weight).T + bias
  input  [8192, 4608] f32, weight [4608, 4608] f32, bias [4608] f32
  BFP: groups of 36 contiguous elements (along rows), shared exponent
  from the group absmax, mantissas truncated toward zero to 8 bits.

Design (v2):
  * Quantization via two custom DVE ops (3 DVE passes total, bit-exact):
      op1 ANT_BFP_FLOORMAG: za = floor(|x|/step)*step using the
          1.5*2^23 magic-constant RNE trick + floor correction (7 ALU ops)
      op2 ANT_COPYSIGN:     q  = za | (x & -0.0), bf16 out (2 ALU ops)
    step = 2^(e-7) comes from absmax-reduce + 2 tiny bit ops per group.
    step=0 (all-zero group) degrades to identity, matching the reference.
  * Sharding: input rows (1024/core) + weight rows (576/core). Each core
    quantizes + PE-transposes its weight shard per K-quarter, AllGathers
    the bf16 [1152, 576] shards into a column-concatenated [1152, 4608]
    view so every o-column block is one contiguous DMA later.
  * Matmul orientation: psum[n=128, o=512]; lhsT = qxT k-tiles (resident
    SBUF), rhs = streamed weight "og-sets" (512 o-columns x 36 k-tiles).
    Output lands as [1024, 4608] per core - no host transpose.
  * Bias rides a K=2 leading matmul per chain: ones[2,128]^T @ [bh;bl]
    where bias = bh + bl splits into two bf16s (error ~4e-8).
  * Pipeline: W-quarters (quant+transpose+AG) -> per-nb: quant x(nb),
    transpose, chains(og0, nb) -> og 1..8 back-to-back chains with
    og-set prefetch via a 48-buf pool.
"""

import numpy as np

import concourse.bass as bass
import concourse.mybir as mybir
import concourse.tile as tile
from concourse import bacc
from concourse import bass_utils
from concourse.masks import make_identity

N_CORES = 8
N_ROWS, K_IN, O_OUT = 8192, 4608, 4608
NSH = N_ROWS // N_CORES   # 1024 input rows per core
OSH = O_OUT // N_CORES    # 576 weight rows per core
GS = 36                   # BFP group size
KT = K_IN // 128          # 36 k tiles
NB = NSH // 128           # 8 n blocks per core
QW = K_IN // 4            # 1152 k per AG quarter = 9 k-tiles
OGW = 512                 # o-columns per matmul chain
NOG = O_OUT // OGW        # 9 o groups
XHW = K_IN // 2           # 2304: x half-tile width for transposes
XQW = QW                  # 1152: x quant chunk width (32 groups)
BIG = 12582912.0          # 1.5 * 2**23

F32 = mybir.dt.float32
BF16 = mybir.dt.bfloat16
I32 = mybir.dt.int32


# --------------------------------------------------------------------------
# Custom DVE ops (registered once per process; additive, name-keyed)
# --------------------------------------------------------------------------

def _register_dve_op(name, spec):
    from concourse import dve_ops as _ops
    from concourse.dve_spec import lower
    from concourse.dve_uop import DveOpSpec

    for op in _ops.OPS:
        if op.name == name:
            return op
    row = 1 + len(_ops.OPS)
    uops = lower(spec, ver="v3")
    sha = DveOpSpec(name=name, opcode=row, uops=uops, rd1_en=True).sha("v3")
    op = _ops.DveOp(name, spec, subdim=False, uops_sha={"v3": sha})
    _ops.OPS.append(op)
    _ops._SUB_OPCODE_FOR_NAME[name] = row
    _ops.CUSTOM_DVE_SPECS[name] = spec
    return op


def register_quant_ops():
    from concourse.dve_spec import Spec, Src0, Src1, C0, C1, Bin
    from concourse.dve_uop import AluOp as A

    ax = Bin(A.ABSOLUTE_VALUE, Src0, Src0)
    b = Src1 * C0                      # C0 = 12582912.0 = 1.5 * 2^23
    y = (ax + b) - b                   # RNE of |x| to step grid (exact)
    c = y > ax
    za = y - c * Src1                  # floor correction

    def _ref1(in0, in1, s0, s1, imm2):
        axn = np.abs(in0).astype(np.float32)
        bn = (in1 * s0).astype(np.float32)
        yn = ((axn + bn) - bn).astype(np.float32)
        return (yn - (yn > axn) * in1).astype(np.float32)

    op1 = _register_dve_op("ANT_BFP_FLOORMAG", Spec(body=za, reference=_ref1))

    sb = Bin(A.BITWISE_AND, Src1, C1)  # C1 = -0.0 -> sign bit of x
    body2 = Bin(A.BITWISE_OR, Src0, sb)

    def _ref2(in0, in1, s0, s1, imm2):
        return np.copysign(in0, in1).astype(np.float32)

    op2 = _register_dve_op("ANT_COPYSIGN", Spec(body=body2, reference=_ref2))
    return op1, op2


# --------------------------------------------------------------------------
# Kernel emission
# --------------------------------------------------------------------------

def _emit_quant(nc, ops, gpool, zapool, src, qdst, rows, width):
    """src[:rows, :width] f32 -> qdst[:rows, :width] bf16 (36-elem groups)."""
    op1, op2 = ops
    g = width // GS
    xg = src[:rows, :width].rearrange("p (g e) -> p g e", e=GS)
    absmax = gpool.tile([128, g], F32, tag="absmax", name="absmax")
    nc.vector.tensor_reduce(
        out=absmax[:rows], in_=xg, axis=mybir.AxisListType.X,
        op=mybir.AluOpType.max, apply_absolute_value=True,
    )
    step = gpool.tile([128, g], F32, tag="step", name="step")
    nc.vector.tensor_scalar(
        out=step[:rows].bitcast(I32), in0=absmax[:rows].bitcast(I32),
        scalar1=0x7F800000, scalar2=None, op0=mybir.AluOpType.bitwise_and,
    )
    nc.vector.tensor_scalar(
        out=step[:rows], in0=step[:rows], scalar1=2.0 ** -7, scalar2=None,
        op0=mybir.AluOpType.mult,
    )
    za = zapool.tile([128, width], F32, tag=f"za{width}", name="za")
    nc.vector._custom_dve(
        op1,
        out=za[:rows].rearrange("p (g e) -> p g e", e=GS),
        in0=xg,
        in1=step[:rows].unsqueeze(-1).broadcast_to([rows, g, GS]),
        s0=BIG,
    )
    nc.vector._custom_dve(
        op2,
        out=qdst[:rows, :width],
        in0=za[:rows],
        in1=src[:rows, :width],
        s1=-0.0,
    )


def emit_kernel(tc, nc, ops, x_d, w_d, b_d, o_d):
    w_tiles = [(i * 128, min(128, OSH - i * 128)) for i in range((OSH + 127) // 128)]
    with (
        tc.tile_pool(name="dram", bufs=1, space="DRAM") as dpool,
        tc.tile_pool(name="consts", bufs=1) as cpool,
        tc.tile_pool(name="grp", bufs=3) as gpool,
        tc.tile_pool(name="za", bufs=2) as zapool,
        tc.tile_pool(name="wstage", bufs=3) as wspool,
        tc.tile_pool(name="qw", bufs=6) as qwpool,
        tc.tile_pool(name="stw", bufs=2) as stwpool,
        tc.tile_pool(name="xstage", bufs=3) as xspool,
        tc.tile_pool(name="qx", bufs=3) as qxpool,
        tc.tile_pool(name="qxT", bufs=1) as xtpool,
        tc.tile_pool(name="ogset", bufs=48) as ogpool,
        tc.tile_pool(name="outs", bufs=3) as opool,
        tc.tile_pool(name="pmm", bufs=4, space="PSUM") as pmm,
        tc.tile_pool(name="ptp", bufs=3, space="PSUM") as ptp,
    ):
        ident = cpool.tile([128, 128], BF16, name="ident")
        make_identity(nc, ident[:])
        ones2 = cpool.tile([2, 128], BF16, name="ones2")
        nc.vector.memset(ones2[:], 1.0)

        # ---------- bias: split into bh + bl (bf16 pair), row layout ----------
        bias_rs = cpool.tile([128, GS], F32, name="bias_rs")
        nc.sync.dma_start(out=bias_rs[:], in_=b_d.rearrange("(p o) -> p o", o=GS))
        bh_rs = cpool.tile([128, GS], BF16, name="bh_rs")
        nc.scalar.copy(bh_rs[:], bias_rs[:])
        bhf_rs = cpool.tile([128, GS], F32, name="bhf_rs")
        nc.scalar.copy(bhf_rs[:], bh_rs[:])
        bl_rs = cpool.tile([128, GS], BF16, name="bl_rs")
        nc.vector.tensor_tensor(
            out=bl_rs[:], in0=bias_rs[:], in1=bhf_rs[:],
            op=mybir.AluOpType.subtract,
        )
        # reshape [128, 36] -> [1, 4608] rows via DRAM bounce
        bh_dr = dpool.tile([O_OUT], BF16, name="bh_dr")
        bl_dr = dpool.tile([O_OUT], BF16, name="bl_dr")
        nc.sync.dma_start(out=bh_dr.rearrange("(p o) -> p o", o=GS), in_=bh_rs[:])
        nc.sync.dma_start(out=bl_dr.rearrange("(p o) -> p o", o=GS), in_=bl_rs[:])
        bias2 = cpool.tile([2, O_OUT], BF16, name="bias2")
        nc.sync.dma_start(out=bias2[0:1, :], in_=bh_dr.rearrange("(a o) -> a o", a=1))
        nc.sync.dma_start(out=bias2[1:2, :], in_=bl_dr.rearrange("(a o) -> a o", a=1))

        # ---------- weight shard: quantize + transpose + bounce + AG ----------
        # qwt_g[q] is [8*QW, OSH]: shard c occupies rows [c*QW, (c+1)*QW).
        qw_boun = [dpool.tile([QW, OSH], BF16, name=f"qw_boun{q}") for q in range(4)]
        qwt_g = [
            dpool.tile([N_CORES * QW, OSH], BF16, addr_space="Shared", name=f"qwt_g{q}")
            for q in range(4)
        ]
        for q in range(4):
            qws = []
            for r0, rows in w_tiles:
                wtile = wspool.tile([128, QW], F32, tag="wstage", name="wtile")
                nc.sync.dma_start(
                    out=wtile[:rows], in_=w_d[r0 : r0 + rows, q * QW : (q + 1) * QW]
                )
                qw = qwpool.tile([128, QW], BF16, tag="qw", name="qw")
                _emit_quant(nc, ops, gpool, zapool, wtile, qw, rows, QW)
                qws.append(qw)
            for ktl in range(9):
                stw = stwpool.tile([128, OSH], BF16, tag="stw", name="stw")
                for (r0, rows), qw in zip(w_tiles, qws):
                    pt = ptp.tile([128, 128], BF16, tag="tp", name="pt")
                    nc.tensor.transpose(
                        pt[:, :rows], qw[:rows, ktl * 128 : (ktl + 1) * 128],
                        ident[:rows, :rows],
                    )
                    nc.scalar.copy(stw[:, r0 : r0 + rows], pt[:, :rows])
                nc.sync.dma_start(
                    out=qw_boun[q][ktl * 128 : (ktl + 1) * 128, :], in_=stw[:]
                )
            nc.gpsimd.collective_compute(
                "AllGather",
                mybir.AluOpType.bypass,
                replica_groups=[list(range(N_CORES))],
                ins=[qw_boun[q][:].opt()],
                outs=[qwt_g[q][:].opt()],
            )

        # ---------- og-set loads (1-2 DMAs per k-tile: shard-span splits) ----
        og_tiles = {}

        def load_ogset(og):
            tl = []
            o0 = og * OGW
            spans = []
            o = o0
            while o < o0 + OGW:
                c = o // OSH
                hi = min((c + 1) * OSH, o0 + OGW)
                spans.append((c, o, hi))
                o = hi
            for kt in range(KT):
                q, ktl = divmod(kt, 9)
                t = ogpool.tile([128, OGW], BF16, tag="og", name=f"og{og}_{kt}")
                for c, lo, hi in spans:
                    # gpsimd queue: keeps the AG-gated triggers off the sync
                    # queue so x/w staging loads are never stuck behind them
                    nc.gpsimd.dma_start(
                        out=t[:, lo - o0 : hi - o0],
                        in_=qwt_g[q][
                            c * QW + ktl * 128 : c * QW + (ktl + 1) * 128,
                            lo - c * OSH : hi - c * OSH,
                        ],
                    )
                tl.append(t)
            og_tiles[og] = tl

        # ---------- x quant (per nb row-tile, 2 half-K chunks) ---------------
        qxT = [xtpool.tile([128, NSH], BF16, name=f"qxT{kt}") for kt in range(KT)]

        def emit_xquant(nb):
            for h in range(2):
                qx = qxpool.tile([128, XHW], BF16, tag="qx", name="qx")
                for ch in range(XHW // XQW):
                    xtile = xspool.tile([128, XQW], F32, tag="xstage", name="xtile")
                    c0 = h * XHW + ch * XQW
                    nc.scalar.dma_start(
                        out=xtile[:],
                        in_=x_d[nb * 128 : (nb + 1) * 128, c0 : c0 + XQW],
                    )
                    _emit_quant(
                        nc, ops, gpool, zapool, xtile,
                        qx[:, ch * XQW : (ch + 1) * XQW], 128, XQW,
                    )
                for ktl in range(KT // 2):
                    kt = h * (KT // 2) + ktl
                    pt = ptp.tile([128, 128], BF16, tag="tp", name="pt")
                    nc.tensor.transpose(
                        pt[:], qx[:, ktl * 128 : (ktl + 1) * 128], ident[:]
                    )
                    nc.scalar.copy(qxT[kt][:, nb * 128 : (nb + 1) * 128], pt[:])

        # ---------- matmul chain: psum[n=128, o=512] -------------------------
        def emit_chain(og, nb):
            ps = pmm.tile([128, OGW], F32, tag="mm", name="ps")
            nc.tensor.matmul(
                ps[:], ones2[:], bias2[:, og * OGW : (og + 1) * OGW],
                start=True, stop=False,
            )
            tl = og_tiles[og]
            for kt in range(KT):
                nc.tensor.matmul(
                    ps[:],
                    qxT[kt][:, nb * 128 : (nb + 1) * 128],
                    tl[kt][:],
                    start=False, stop=(kt == KT - 1),
                )
            ot = opool.tile([128, OGW], F32, tag="ot", name="ot")
            nc.scalar.copy(ot[:], ps[:])
            nc.scalar.dma_start(
                out=o_d[nb * 128 : (nb + 1) * 128, og * OGW : (og + 1) * OGW],
                in_=ot[:],
            )

        # ---------- schedule ------------------------------------------------
        load_ogset(0)
        for nb in range(NB):
            emit_xquant(nb)
            emit_chain(0, nb)
        for og in range(1, NOG):
            load_ogset(og)
            for nb in range(NB):
                emit_chain(og, nb)
            del og_tiles[og - 1]


_CACHED_NC = None


def _build():
    global _CACHED_NC
    if _CACHED_NC is not None:
        return _CACHED_NC
    ops = register_quant_ops()
    nc = bacc.Bacc(
        "TRN2", target_bir_lowering=False, debug=False, num_devices=N_CORES
    )
    x_d = nc.dram_tensor("x", [NSH, K_IN], F32, kind="ExternalInput").ap()
    w_d = nc.dram_tensor("w", [OSH, K_IN], F32, kind="ExternalInput").ap()
    b_d = nc.dram_tensor("b", [O_OUT], F32, kind="ExternalInput").ap()
    o_d = nc.dram_tensor("o", [NSH, O_OUT], F32, kind="ExternalOutput").ap()
    with tile.TileContext(nc) as tc:
        emit_kernel(tc, nc, ops, x_d, w_d, b_d, o_d)
    nc.compile()
    _CACHED_NC = nc
    return nc


def _ensure_axon_hooks_importable():
    # bass_utils imports antenv.axon_hooks when tracing is requested; the
    # slim agent image lacks it. Provide a no-op so a stray BASS_TRACE env
    # degrades to "no trace" instead of crashing.
    import sys
    import types

    if "antenv.axon_hooks" not in sys.modules:
        try:
            import antenv.axon_hooks  # noqa: F401
        except ImportError:
            mod = types.ModuleType("antenv.axon_hooks")
            mod.get_axon_ntff_profile_hook = lambda: None
            mod.set_axon_ntff_profile_hook = lambda h: None
            sys.modules["antenv.axon_hooks"] = mod


def run_on_hw(input, weight, bias, trace=False):
    _ensure_axon_hooks_importable()
    nc = _build()
    in_maps = []
    for c in range(N_CORES):
        in_maps.append(
            {
                "x": np.ascontiguousarray(input[c * NSH : (c + 1) * NSH]),
                "w": np.ascontiguousarray(weight[c * OSH : (c + 1) * OSH]),
                "b": np.ascontiguousarray(bias),
            }
        )
    res = bass_utils.run_bass_kernel_spmd(
        nc, in_maps, core_ids=list(range(N_CORES)), trace=trace
    )
    out = np.empty((N_ROWS, O_OUT), dtype=np.float32)
    for c in range(N_CORES):
        out[c * NSH : (c + 1) * NSH] = res.results[c]["o"]
    return out, res


def kernel(input, weight, bias):
    out, _ = run_on_hw(
        np.asarray(input, dtype=np.float32),
        np.asarray(weight, dtype=np.float32),
        np.asarray(bias, dtype=np.float32),
    )
    return out


# revision 15
# speedup vs baseline: 1.2135x; 1.0189x over previous
"""BFP-quantized linear kernel for Trainium2, 8-core SPMD.

out = bfp_quantize(input) @ bfp_quantize(weight).T + bias
  input  [8192, 4608] f32, weight [4608, 4608] f32, bias [4608] f32
  BFP: groups of 36 contiguous elements (along rows), shared exponent
  from the group absmax, mantissas truncated toward zero to 8 bits.

Design (v2):
  * Quantization via two custom DVE ops (3 DVE passes total, bit-exact):
      op1 ANT_BFP_FLOORMAG: za = floor(|x|/step)*step using the
          1.5*2^23 magic-constant RNE trick + floor correction (7 ALU ops)
      op2 ANT_COPYSIGN:     q  = za | (x & -0.0), bf16 out (2 ALU ops)
    step = 2^(e-7) comes from absmax-reduce + 2 tiny bit ops per group.
    step=0 (all-zero group) degrades to identity, matching the reference.
  * Sharding: input rows (1024/core) + weight rows (576/core). Each core
    quantizes + PE-transposes its weight shard per K-quarter, AllGathers
    the bf16 [1152, 576] shards into a column-concatenated [1152, 4608]
    view so every o-column block is one contiguous DMA later.
  * Matmul orientation: psum[n=128, o=512]; lhsT = qxT k-tiles (resident
    SBUF), rhs = streamed weight "og-sets" (512 o-columns x 36 k-tiles).
    Output lands as [1024, 4608] per core - no host transpose.
  * Bias rides a K=2 leading matmul per chain: ones[2,128]^T @ [bh;bl]
    where bias = bh + bl splits into two bf16s (error ~4e-8).
  * Pipeline: W-quarters (quant+transpose+AG) -> per-nb: quant x(nb),
    transpose, chains(og0, nb) -> og 1..8 back-to-back chains with
    og-set prefetch via a 48-buf pool.
"""

import numpy as np

import concourse.bass as bass
import concourse.mybir as mybir
import concourse.tile as tile
from concourse import bacc
from concourse import bass_utils
from concourse.masks import make_identity

N_CORES = 8
N_ROWS, K_IN, O_OUT = 8192, 4608, 4608
NSH = N_ROWS // N_CORES   # 1024 input rows per core
OSH = O_OUT // N_CORES    # 576 weight rows per core
GS = 36                   # BFP group size
KT = K_IN // 128          # 36 k tiles
NB = NSH // 128           # 8 n blocks per core
QW = K_IN // 4            # 1152 k per AG quarter = 9 k-tiles
OGW = 512                 # o-columns per matmul chain
NOG = O_OUT // OGW        # 9 o groups
XHW = K_IN // 2           # 2304: x half-tile width for transposes
XQW = QW                  # 1152: x quant chunk width (32 groups)
BIG = 12582912.0          # 1.5 * 2**23

F32 = mybir.dt.float32
BF16 = mybir.dt.bfloat16
I32 = mybir.dt.int32


# --------------------------------------------------------------------------
# Custom DVE ops (registered once per process; additive, name-keyed)
# --------------------------------------------------------------------------

def _register_dve_op(name, spec):
    from concourse import dve_ops as _ops
    from concourse.dve_spec import lower
    from concourse.dve_uop import DveOpSpec

    for op in _ops.OPS:
        if op.name == name:
            return op
    row = 1 + len(_ops.OPS)
    uops = lower(spec, ver="v3")
    sha = DveOpSpec(name=name, opcode=row, uops=uops, rd1_en=True).sha("v3")
    op = _ops.DveOp(name, spec, subdim=False, uops_sha={"v3": sha})
    _ops.OPS.append(op)
    _ops._SUB_OPCODE_FOR_NAME[name] = row
    _ops.CUSTOM_DVE_SPECS[name] = spec
    return op


def register_quant_ops():
    from concourse.dve_spec import Spec, Src0, Src1, C0, C1, Bin
    from concourse.dve_uop import AluOp as A

    ax = Bin(A.ABSOLUTE_VALUE, Src0, Src0)
    b = Src1 * C0                      # C0 = 12582912.0 = 1.5 * 2^23
    y = (ax + b) - b                   # RNE of |x| to step grid (exact)
    c = y > ax
    za = y - c * Src1                  # floor correction

    def _ref1(in0, in1, s0, s1, imm2):
        axn = np.abs(in0).astype(np.float32)
        bn = (in1 * s0).astype(np.float32)
        yn = ((axn + bn) - bn).astype(np.float32)
        return (yn - (yn > axn) * in1).astype(np.float32)

    op1 = _register_dve_op("ANT_BFP_FLOORMAG", Spec(body=za, reference=_ref1))

    sb = Bin(A.BITWISE_AND, Src1, C1)  # C1 = -0.0 -> sign bit of x
    body2 = Bin(A.BITWISE_OR, Src0, sb)

    def _ref2(in0, in1, s0, s1, imm2):
        return np.copysign(in0, in1).astype(np.float32)

    op2 = _register_dve_op("ANT_COPYSIGN", Spec(body=body2, reference=_ref2))
    return op1, op2


# --------------------------------------------------------------------------
# Kernel emission
# --------------------------------------------------------------------------

def _emit_quant(nc, ops, gpool, zapool, src, qdst, rows, width):
    """src[:rows, :width] f32 -> qdst[:rows, :width] bf16 (36-elem groups)."""
    op1, op2 = ops
    g = width // GS
    xg = src[:rows, :width].rearrange("p (g e) -> p g e", e=GS)
    absmax = gpool.tile([128, g], F32, tag="absmax", name="absmax")
    nc.vector.tensor_reduce(
        out=absmax[:rows], in_=xg, axis=mybir.AxisListType.X,
        op=mybir.AluOpType.max, apply_absolute_value=True,
    )
    step = gpool.tile([128, g], F32, tag="step", name="step")
    nc.vector.tensor_scalar(
        out=step[:rows].bitcast(I32), in0=absmax[:rows].bitcast(I32),
        scalar1=0x7F800000, scalar2=None, op0=mybir.AluOpType.bitwise_and,
    )
    nc.vector.tensor_scalar(
        out=step[:rows], in0=step[:rows], scalar1=2.0 ** -7, scalar2=None,
        op0=mybir.AluOpType.mult,
    )
    za = zapool.tile([128, width], F32, tag=f"za{width}", name="za")
    nc.vector._custom_dve(
        op1,
        out=za[:rows].rearrange("p (g e) -> p g e", e=GS),
        in0=xg,
        in1=step[:rows].unsqueeze(-1).broadcast_to([rows, g, GS]),
        s0=BIG,
    )
    nc.vector._custom_dve(
        op2,
        out=qdst[:rows, :width],
        in0=za[:rows],
        in1=src[:rows, :width],
        s1=-0.0,
    )


def emit_kernel(tc, nc, ops, x_d, w_d, b_d, o_d):
    w_tiles = [(i * 128, min(128, OSH - i * 128)) for i in range((OSH + 127) // 128)]
    with (
        tc.tile_pool(name="dram", bufs=1, space="DRAM") as dpool,
        tc.tile_pool(name="consts", bufs=1) as cpool,
        tc.tile_pool(name="grp", bufs=3) as gpool,
        tc.tile_pool(name="za", bufs=2) as zapool,
        tc.tile_pool(name="wstage", bufs=3) as wspool,
        tc.tile_pool(name="qw", bufs=6) as qwpool,
        tc.tile_pool(name="stw", bufs=2) as stwpool,
        tc.tile_pool(name="xstage", bufs=3) as xspool,
        tc.tile_pool(name="qx", bufs=3) as qxpool,
        tc.tile_pool(name="qxT", bufs=1) as xtpool,
        tc.tile_pool(name="ogset", bufs=48) as ogpool,
        tc.tile_pool(name="outs", bufs=3) as opool,
        tc.tile_pool(name="pmm", bufs=4, space="PSUM") as pmm,
        tc.tile_pool(name="ptp", bufs=3, space="PSUM") as ptp,
    ):
        ident = cpool.tile([128, 128], BF16, name="ident")
        make_identity(nc, ident[:])
        ones2 = cpool.tile([2, 128], BF16, name="ones2")
        nc.vector.memset(ones2[:], 1.0)

        # ---------- bias: split into bh + bl (bf16 pair), row layout ----------
        bias_rs = cpool.tile([128, GS], F32, name="bias_rs")
        nc.sync.dma_start(out=bias_rs[:], in_=b_d.rearrange("(p o) -> p o", o=GS))
        bh_rs = cpool.tile([128, GS], BF16, name="bh_rs")
        nc.scalar.copy(bh_rs[:], bias_rs[:])
        bhf_rs = cpool.tile([128, GS], F32, name="bhf_rs")
        nc.scalar.copy(bhf_rs[:], bh_rs[:])
        bl_rs = cpool.tile([128, GS], BF16, name="bl_rs")
        nc.vector.tensor_tensor(
            out=bl_rs[:], in0=bias_rs[:], in1=bhf_rs[:],
            op=mybir.AluOpType.subtract,
        )
        # reshape [128, 36] -> [1, 4608] rows via DRAM bounce
        bh_dr = dpool.tile([O_OUT], BF16, name="bh_dr")
        bl_dr = dpool.tile([O_OUT], BF16, name="bl_dr")
        nc.sync.dma_start(out=bh_dr.rearrange("(p o) -> p o", o=GS), in_=bh_rs[:])
        nc.sync.dma_start(out=bl_dr.rearrange("(p o) -> p o", o=GS), in_=bl_rs[:])
        bias2 = cpool.tile([2, O_OUT], BF16, name="bias2")
        nc.sync.dma_start(out=bias2[0:1, :], in_=bh_dr.rearrange("(a o) -> a o", a=1))
        nc.sync.dma_start(out=bias2[1:2, :], in_=bl_dr.rearrange("(a o) -> a o", a=1))

        # ---------- weight shard: quantize + transpose + bounce + AG ----------
        # qwt_g[q] is [8*QW, OSH]: shard c occupies rows [c*QW, (c+1)*QW).
        qw_boun = [dpool.tile([QW, OSH], BF16, name=f"qw_boun{q}") for q in range(4)]
        qwt_g = [
            dpool.tile([N_CORES * QW, OSH], BF16, addr_space="Shared", name=f"qwt_g{q}")
            for q in range(4)
        ]
        for q in range(4):
            qws = []
            for r0, rows in w_tiles:
                wtile = wspool.tile([128, QW], F32, tag="wstage", name="wtile")
                nc.sync.dma_start(
                    out=wtile[:rows], in_=w_d[r0 : r0 + rows, q * QW : (q + 1) * QW]
                )
                qw = qwpool.tile([128, QW], BF16, tag="qw", name="qw")
                _emit_quant(nc, ops, gpool, zapool, wtile, qw, rows, QW)
                qws.append(qw)
            for ktl in range(9):
                stw = stwpool.tile([128, OSH], BF16, tag="stw", name="stw")
                for (r0, rows), qw in zip(w_tiles, qws):
                    pt = ptp.tile([128, 128], BF16, tag="tp", name="pt")
                    nc.tensor.transpose(
                        pt[:, :rows], qw[:rows, ktl * 128 : (ktl + 1) * 128],
                        ident[:rows, :rows],
                    )
                    nc.scalar.copy(stw[:, r0 : r0 + rows], pt[:, :rows])
                nc.sync.dma_start(
                    out=qw_boun[q][ktl * 128 : (ktl + 1) * 128, :], in_=stw[:]
                )
            nc.gpsimd.collective_compute(
                "AllGather",
                mybir.AluOpType.bypass,
                replica_groups=[list(range(N_CORES))],
                ins=[qw_boun[q][:].opt()],
                outs=[qwt_g[q][:].opt()],
            )

        # ---------- og-set loads (1-2 DMAs per k-tile: shard-span splits) ----
        og_tiles = {}

        def load_ogset(og):
            tl = []
            o0 = og * OGW
            spans = []
            o = o0
            while o < o0 + OGW:
                c = o // OSH
                hi = min((c + 1) * OSH, o0 + OGW)
                spans.append((c, o, hi))
                o = hi
            for kt in range(KT):
                q, ktl = divmod(kt, 9)
                t = ogpool.tile([128, OGW], BF16, tag="og", name=f"og{og}_{kt}")
                for c, lo, hi in spans:
                    # gpsimd queue: keeps the AG-gated triggers off the sync
                    # queue so x/w staging loads are never stuck behind them
                    nc.gpsimd.dma_start(
                        out=t[:, lo - o0 : hi - o0],
                        in_=qwt_g[q][
                            c * QW + ktl * 128 : c * QW + (ktl + 1) * 128,
                            lo - c * OSH : hi - c * OSH,
                        ],
                    )
                tl.append(t)
            og_tiles[og] = tl

        # ---------- x quant (per nb row-tile, 2 half-K chunks) ---------------
        qxT = [xtpool.tile([128, NSH], BF16, name=f"qxT{kt}") for kt in range(KT)]

        def emit_xquant(nb):
            for h in range(2):
                qx = qxpool.tile([128, XHW], BF16, tag="qx", name="qx")
                for ch in range(XHW // XQW):
                    xtile = xspool.tile([128, XQW], F32, tag="xstage", name="xtile")
                    c0 = h * XHW + ch * XQW
                    nc.scalar.dma_start(
                        out=xtile[:],
                        in_=x_d[nb * 128 : (nb + 1) * 128, c0 : c0 + XQW],
                    )
                    _emit_quant(
                        nc, ops, gpool, zapool, xtile,
                        qx[:, ch * XQW : (ch + 1) * XQW], 128, XQW,
                    )
                for ktl in range(KT // 2):
                    kt = h * (KT // 2) + ktl
                    pt = ptp.tile([128, 128], BF16, tag="tp", name="pt")
                    nc.tensor.transpose(
                        pt[:], qx[:, ktl * 128 : (ktl + 1) * 128], ident[:]
                    )
                    nc.scalar.copy(qxT[kt][:, nb * 128 : (nb + 1) * 128], pt[:])

        # ---------- matmul chains: psum[n=128, o=512] ------------------------
        def bias_mm(ps, og):
            nc.tensor.matmul(
                ps[:], ones2[:], bias2[:, og * OGW : (og + 1) * OGW],
                start=True, stop=False,
            )

        def seg(og, nb, ps, q, last=False):
            """Accumulate k-quarter q of chain (og, nb) into held psum ps."""
            tl = og_tiles[og]
            for kt in range(q * 9, (q + 1) * 9):
                nc.tensor.matmul(
                    ps[:],
                    qxT[kt][:, nb * 128 : (nb + 1) * 128],
                    tl[kt][:],
                    start=False, stop=(last and kt == (q + 1) * 9 - 1),
                )

        def drain(og, nb, ps):
            ot = opool.tile([128, OGW], F32, tag="ot", name="ot")
            nc.scalar.copy(ot[:], ps[:])
            nc.scalar.dma_start(
                out=o_d[nb * 128 : (nb + 1) * 128, og * OGW : (og + 1) * OGW],
                in_=ot[:],
            )

        def emit_chain_pair(og, nbA, nbB):
            """Two chains interleaved MM-by-MM (breaks accumulation serial
            dependency: ~220 vs ~231 ns cadence)."""
            psA = pmm.tile([128, OGW], F32, tag="mm", name="psA")
            psB = pmm.tile([128, OGW], F32, tag="mm", name="psB")
            bias_mm(psA, og)
            bias_mm(psB, og)
            tl = og_tiles[og]
            for kt in range(KT):
                for nb, ps in ((nbA, psA), (nbB, psB)):
                    nc.tensor.matmul(
                        ps[:],
                        qxT[kt][:, nb * 128 : (nb + 1) * 128],
                        tl[kt][:],
                        start=False, stop=(kt == KT - 1),
                    )
            drain(og, nbA, psA)
            drain(og, nbB, psB)

        # ---------- schedule ------------------------------------------------
        # og0's nb0-3 chains are segmented by k-quarter and interleaved with
        # the x quant batches, so the PE consumes AG quarters as they land
        # without blocking the x transposes queued behind them.
        emit_xquant(0)
        emit_xquant(1)
        load_ogset(0)
        ps0 = {}
        for nb in range(4):
            ps0[nb] = pmm.tile([128, OGW], F32, tag="mm", name=f"ps0_{nb}")
        bias_mm(ps0[0], 0); seg(0, 0, ps0[0], 0)
        bias_mm(ps0[1], 0); seg(0, 1, ps0[1], 0)
        emit_xquant(2)
        emit_xquant(3)
        bias_mm(ps0[2], 0); seg(0, 2, ps0[2], 0)
        bias_mm(ps0[3], 0); seg(0, 3, ps0[3], 0)
        seg(0, 0, ps0[0], 1); seg(0, 1, ps0[1], 1)
        emit_xquant(4)
        emit_xquant(5)
        seg(0, 2, ps0[2], 1); seg(0, 3, ps0[3], 1)
        seg(0, 0, ps0[0], 2); seg(0, 1, ps0[1], 2)
        emit_xquant(6)
        emit_xquant(7)
        seg(0, 2, ps0[2], 2); seg(0, 3, ps0[3], 2)
        for nb in range(4):
            seg(0, nb, ps0[nb], 3, last=True)
            drain(0, nb, ps0[nb])
        emit_chain_pair(0, 4, 5)
        emit_chain_pair(0, 6, 7)
        for og in range(1, NOG):
            load_ogset(og)
            for a, b in ((0, 1), (2, 3), (4, 5), (6, 7)):
                emit_chain_pair(og, a, b)
            del og_tiles[og - 1]


_CACHED_NC = None


def _build():
    global _CACHED_NC
    if _CACHED_NC is not None:
        return _CACHED_NC
    ops = register_quant_ops()
    nc = bacc.Bacc(
        "TRN2", target_bir_lowering=False, debug=False, num_devices=N_CORES
    )
    x_d = nc.dram_tensor("x", [NSH, K_IN], F32, kind="ExternalInput").ap()
    w_d = nc.dram_tensor("w", [OSH, K_IN], F32, kind="ExternalInput").ap()
    b_d = nc.dram_tensor("b", [O_OUT], F32, kind="ExternalInput").ap()
    o_d = nc.dram_tensor("o", [NSH, O_OUT], F32, kind="ExternalOutput").ap()
    with tile.TileContext(nc) as tc:
        emit_kernel(tc, nc, ops, x_d, w_d, b_d, o_d)
    nc.compile()
    _CACHED_NC = nc
    return nc


def _ensure_axon_hooks_importable():
    # bass_utils imports antenv.axon_hooks when tracing is requested; the
    # slim agent image lacks it. Provide a no-op so a stray BASS_TRACE env
    # degrades to "no trace" instead of crashing.
    import sys
    import types

    if "antenv.axon_hooks" not in sys.modules:
        try:
            import antenv.axon_hooks  # noqa: F401
        except ImportError:
            mod = types.ModuleType("antenv.axon_hooks")
            mod.get_axon_ntff_profile_hook = lambda: None
            mod.set_axon_ntff_profile_hook = lambda h: None
            sys.modules["antenv.axon_hooks"] = mod


def run_on_hw(input, weight, bias, trace=False):
    _ensure_axon_hooks_importable()
    nc = _build()
    in_maps = []
    for c in range(N_CORES):
        in_maps.append(
            {
                "x": np.ascontiguousarray(input[c * NSH : (c + 1) * NSH]),
                "w": np.ascontiguousarray(weight[c * OSH : (c + 1) * OSH]),
                "b": np.ascontiguousarray(bias),
            }
        )
    res = bass_utils.run_bass_kernel_spmd(
        nc, in_maps, core_ids=list(range(N_CORES)), trace=trace
    )
    out = np.empty((N_ROWS, O_OUT), dtype=np.float32)
    for c in range(N_CORES):
        out[c * NSH : (c + 1) * NSH] = res.results[c]["o"]
    return out, res


def kernel(input, weight, bias):
    out, _ = run_on_hw(
        np.asarray(input, dtype=np.float32),
        np.asarray(weight, dtype=np.float32),
        np.asarray(bias, dtype=np.float32),
    )
    return out
